# revision 11
# baseline (speedup 1.0000x reference)
"""Trainium2 Bass kernel for the DSConv1d block (relu -> BN(eval) -> depthwise
conv1d(k=3,pad=1) -> PReLU -> GlobalLayerNorm -> pointwise conv -> residual).

Sharding: data-parallel over batch B=16 across 8 NeuronCores (2 samples/core).
Everything per-sample is device-local; no collectives.

v2 design (vs baseline): bf16 end-to-end on device (halves DMA + enables DVE
2x/4x modes), two-sample pipelining with issue order phase1(b0), phase1(b1),
stats(b0), pw(b0), stats(b1), pw(b1) so the gLN stats reduction of sample b
hides under other-sample PE work (PE never stalls at the stats barrier), and
a tunable split of the depthwise conv between the PE (diagonal matmuls) and
the DVE (ratio-trick MACs) to balance engine load.

NOTE: tensor_tensor_reduce and partition_all_reduce crash this HW stack —
use mul+reduce and ones-matmul+partition_broadcast instead.

Depthwise conv on DVE (per [128,2000] tile, all bf16):
  t0  = relu(x)*(s*w0)    with 1-col halos     (tensor_scalar max+mult)
  c0  = relu(x)*(s*w1)                          (tensor_scalar, 4x mode)
  a1  = t0[+2]*(w2/w0) + t0[+0]                 (scalar_tensor_tensor, 2x)
  acc = a1 + c0                                 (tensor_tensor, 2x)
then PReLU on ACT with bias bsum = bb*(w0+w1+w2), like the PE path.
"""

import numpy as np

B, C, T = 16, 512, 4000
NCORES = 8
BPC = B // NCORES          # samples per core
CT = 4                     # channel tiles of 128
P = 128
TH = 2                     # halves of T
HW_ = T // TH              # 2000
CHUNKS = [(0, 512), (512, 512), (1024, 512), (1536, 464)]  # bank-aligned
BN_EPS = 1e-5
GLN_EPS = 1e-8
INVN = 1.0 / float(C * T)

# number of (ci,h) tiles per sample whose depthwise conv runs on the DVE
# instead of the PE (0..8); flip order chosen to keep early tiles on PE.
N_DVE = 0
DVE_ORDER = [(3, 1), (1, 1), (2, 1), (0, 1), (3, 0), (1, 0), (2, 0), (0, 0)]

_CACHE = {}


def _build(alpha: float, n_dve: int = N_DVE):
    import concourse.bass as bass
    import concourse.mybir as mybir
    import concourse.tile as tile
    from concourse import bacc, bass_isa

    f32 = mybir.dt.float32
    bf16 = mybir.dt.bfloat16
    AF = mybir.ActivationFunctionType
    OP = mybir.AluOpType
    AX = mybir.AxisListType

    dve_set = set(DVE_ORDER[:n_dve])

    nc = bacc.Bacc("TRN2", target_bir_lowering=False, debug=False)

    x_d = nc.dram_tensor("x", [BPC, C, T], bf16, kind="ExternalInput")
    # batched constants: one DMA each (a dma_start has ~2us fixed cost)
    dg_d = nc.dram_tensor("dg", [P, CT * 3 * P], bf16, kind="ExternalInput")
    wt_d = nc.dram_tensor("wt", [P, CT * C], bf16, kind="ExternalInput")
    cvb_d = nc.dram_tensor("cvb", [P, 2 * CT], bf16, kind="ExternalInput")
    cvf_d = nc.dram_tensor("cvf", [P, 6 * CT], f32, kind="ExternalInput")
    y_d = nc.dram_tensor("y", [BPC, C, T], bf16, kind="ExternalOutput")

    with tile.TileContext(nc) as tc:
        with (
            tc.tile_pool(name="cpool", bufs=1) as cpool,
            tc.tile_pool(name="xpool", bufs=8) as xpool,
            tc.tile_pool(name="ppool", bufs=8) as ppool,
            tc.tile_pool(name="gpool", bufs=3) as gpool,
            tc.tile_pool(name="dpool", bufs=2) as dpool,
            tc.tile_pool(name="opool", bufs=3) as opool,
            tc.tile_pool(name="wscp", bufs=4) as wscp,
            tc.tile_pool(name="spool", bufs=2) as spool,
            tc.tile_pool(name="pspool", bufs=2, space=bass.MemorySpace.PSUM) as pspool,
        ):
            # ---- constants (4 batched DMAs) ----
            dg_sb = cpool.tile([P, CT * 3 * P], bf16, tag="dg", name="dg_sb")
            nc.sync.dma_start(dg_sb[:], dg_d[:])
            wt_all = cpool.tile([P, CT * C], bf16, tag="wt", name="wt_all")
            cvb = cpool.tile([P, 2 * CT], bf16, tag="cvb", name="cvb")
            nc.sync.dma_start(cvb[:], cvb_d[:])
            cvf = cpool.tile([P, 6 * CT], f32, tag="cvf", name="cvf")
            nc.sync.dma_start(cvf[:], cvf_d[:])

            def dgv(ci, k):
                j = (ci * 3 + k) * P
                return dg_sb[:, j:j + P]

            def wtv(k):
                return wt_all[:, k * C:(k + 1) * C]

            pads = cvb[:, 0:CT]
            pads0 = cvb[:, CT:2 * CT]
            bsum = cvf[:, 0:CT]
            w0s = cvf[:, CT:2 * CT]
            w1s = cvf[:, 2 * CT:3 * CT]
            r2c = cvf[:, 3 * CT:4 * CT]
            wgam = cvf[:, 4 * CT:5 * CT]
            wbet = cvf[:, 5 * CT:6 * CT]
            ones = cpool.tile([P, 1], f32, tag="ones", name="ones")
            nc.vector.memset(ones[:], 1.0)

            xt = {}
            pt = {}
            d_b = {}
            wsc = {}
            sums_b = {}

            def emit_sq(bs, ids, eng):
                o0s = (ids % TH) * HW_
                cis = ids // TH
                psl = pt[bs, cis][:, o0s:o0s + HW_]
                sq = dpool.tile([P, HW_], bf16, tag="sqs", bufs=2,
                                name=f"sq{bs}{ids}{eng}")
                if eng == "v":
                    nc.vector.tensor_mul(sq[:], psl, psl)
                    nc.vector.tensor_reduce(
                        sums_b[bs][:, 8 + ids:9 + ids], sq[:], AX.X, OP.add)
                else:
                    nc.scalar.activation(
                        sq[:], psl, AF.Square,
                        accum_out=sums_b[bs][:, 8 + ids:9 + ids])

            # sq pass schedule: (b_loop, tile_idx_or_end) -> [(b_src, idx, eng)]
            SQ_PLAN = {
                (0, 2): [(0, 0, "v")],
                (0, 4): [(0, 2, "a")],
                (0, 6): [(0, 4, "v")],
                (1, 0): [(0, 6, "a")],
                (1, 1): [(0, 1, "v")],
                (1, 2): [(0, 3, "a")],
                (1, 3): [(0, 5, "v")],
                (1, 4): [(0, 7, "a")],
                (1, "end"): [(1, i, "a") for i in range(8)],
            }

            # ---------------- phase 1 (per sample) ----------------
            for b in range(BPC):
                for ci in range(CT):
                    t = xpool.tile([P, T], bf16, tag="x", name=f"x{b}{ci}")
                    nc.sync.dma_start(t[:], x_d[b, ci * P:(ci + 1) * P, :])
                    xt[b, ci] = t
                    pt[b, ci] = ppool.tile([P, T], bf16, tag="p", name=f"p{b}{ci}")
                if b == 0:
                    # weights DMA after b0's x tiles so the first conv starts asap
                    nc.sync.dma_start(wt_all[:], wt_d[:])
                sums = spool.tile([P, 16], f32, tag="sums", name=f"sums{b}")
                sums_b[b] = sums

                for ci in range(CT):
                    for h in range(TH):
                        idx = ci * TH + h
                        o0 = h * HW_
                        xs = xt[b, ci]
                        if (ci, h) not in dve_set:
                            # ---- depthwise conv on PE ----
                            # g[:, j] = relu(x[o0-1+j]); pads at sample edges
                            g = gpool.tile([P, HW_ + 2], bf16, tag="g",
                                           name=f"g{b}{idx}")
                            nc.vector.tensor_scalar_max(
                                g[:, 1:HW_ + 1], xs[:, o0:o0 + HW_], 0.0)
                            if h == 0:
                                nc.vector.tensor_copy(g[:, 0:1],
                                                      pads[:, ci:ci + 1])
                            else:
                                nc.vector.tensor_scalar_max(
                                    g[:, 0:1], xs[:, o0 - 1:o0], 0.0)
                            if h == TH - 1:
                                nc.vector.tensor_copy(g[:, HW_ + 1:HW_ + 2],
                                                      pads[:, ci:ci + 1])
                            else:
                                nc.vector.tensor_scalar_max(
                                    g[:, HW_ + 1:HW_ + 2],
                                    xs[:, o0 + HW_:o0 + HW_ + 1], 0.0)
                            cps = pspool.tile([P, 2048], f32, tag="ps",
                                              name=f"cps{b}{idx}")
                            for k in range(3):
                                for c0, wc in CHUNKS:
                                    nc.tensor.matmul(
                                        cps[:, c0:c0 + wc],
                                        dgv(ci, k),
                                        g[:, k + c0: k + c0 + wc],
                                        start=(k == 0), stop=(k == 2))
                            nc.scalar.activation(
                                pt[b, ci][:, o0:o0 + HW_], cps[:, 0:HW_],
                                AF.Prelu, bias=bsum[:, ci:ci + 1], scale=1.0,
                                alpha=alpha,
                                accum_out=sums[:, idx:idx + 1])
                        else:
                            # ---- depthwise conv on DVE (ratio trick) ----
                            t0 = gpool.tile([P, HW_ + 2], bf16, tag="g",
                                            name=f"t0{b}{idx}")
                            nc.vector.tensor_scalar(
                                t0[:, 1:HW_ + 1], xs[:, o0:o0 + HW_],
                                0.0, w0s[:, ci:ci + 1], OP.max, OP.mult)
                            if h == 0:
                                nc.vector.tensor_copy(t0[:, 0:1],
                                                      pads0[:, ci:ci + 1])
                            else:
                                nc.vector.tensor_scalar(
                                    t0[:, 0:1], xs[:, o0 - 1:o0],
                                    0.0, w0s[:, ci:ci + 1], OP.max, OP.mult)
                            if h == TH - 1:
                                nc.vector.tensor_copy(t0[:, HW_ + 1:HW_ + 2],
                                                      pads0[:, ci:ci + 1])
                            else:
                                nc.vector.tensor_scalar(
                                    t0[:, HW_ + 1:HW_ + 2],
                                    xs[:, o0 + HW_:o0 + HW_ + 1],
                                    0.0, w0s[:, ci:ci + 1], OP.max, OP.mult)
                            c0t = dpool.tile([P, HW_], bf16, tag="c0",
                                             name=f"c0{b}{idx}")
                            nc.vector.tensor_scalar(
                                c0t[:], xs[:, o0:o0 + HW_],
                                0.0, w1s[:, ci:ci + 1], OP.max, OP.mult)
                            a1t = dpool.tile([P, HW_], bf16, tag="a1",
                                             name=f"a1{b}{idx}")
                            nc.vector.scalar_tensor_tensor(
                                a1t[:], t0[:, 2:HW_ + 2], r2c[:, ci:ci + 1],
                                t0[:, 0:HW_], OP.mult, OP.add)
                            # acc overwrites t0's buffer region (t0 is dead)
                            nc.vector.tensor_add(t0[:, 0:HW_], a1t[:], c0t[:])
                            nc.scalar.activation(
                                pt[b, ci][:, o0:o0 + HW_], t0[:, 0:HW_],
                                AF.Prelu, bias=bsum[:, ci:ci + 1], scale=1.0,
                                alpha=alpha,
                                accum_out=sums[:, idx:idx + 1])
                        # squared-sum passes are emitted via sq_plan to
                        # balance ACT/DVE load against the PE conv stream
                        for bs, ids, eng in SQ_PLAN.get((b, idx), ()):
                            emit_sq(bs, ids, eng)
                for bs, ids, eng in SQ_PLAN.get((b, "end"), ()):
                    emit_sq(bs, ids, eng)

            # ---------- stats(b) + pointwise(b), interleaved issue ----------
            for b in range(BPC):
                # cross-partition reduce via ones-matmul into a shared-ring
                # PSUM slot, scalar chain on partition 0, then broadcast
                spr = pspool.tile([P, 2048], f32, tag="ps", name=f"spr{b}")
                nc.tensor.matmul(spr[0:1, 0:16], ones[:], sums_b[b][:],
                                 start=True, stop=True)
                st = spool.tile([1, 13], f32, tag="st", name=f"st{b}")
                iS, iQ, iMEAN, iE2, iMSQ, iVAR, iA, iS0, iR0, iAR, iS1, \
                    iRSTD, iRM = range(13)

                def stc(i):
                    return st[0:1, i:i + 1]

                nc.vector.tensor_reduce(stc(iS), spr[0:1, 0:8], AX.X, OP.add)
                nc.vector.tensor_reduce(stc(iQ), spr[0:1, 8:16], AX.X, OP.add)
                nc.vector.tensor_scalar_mul(stc(iMEAN), stc(iS), INVN)
                nc.vector.tensor_scalar_mul(stc(iE2), stc(iQ), INVN)
                nc.vector.tensor_scalar(stc(iMSQ), stc(iMEAN), stc(iMEAN),
                                        None, OP.mult)
                nc.vector.scalar_tensor_tensor(stc(iVAR), stc(iMSQ), -1.0,
                                               stc(iE2), OP.mult, OP.add)
                nc.vector.tensor_scalar_add(stc(iA), stc(iVAR), GLN_EPS)
                nc.scalar.activation(stc(iS0), stc(iA), AF.Sqrt)
                nc.vector.reciprocal(stc(iR0), stc(iS0))
                # one Newton step for sqrt: s1 = 0.5*(s0 + a*r0)
                nc.vector.tensor_scalar(stc(iAR), stc(iA), stc(iR0), None,
                                        OP.mult)
                nc.vector.tensor_scalar(stc(iS1), stc(iAR), stc(iS0), 0.5,
                                        OP.add, OP.mult)
                nc.vector.reciprocal(stc(iRSTD), stc(iS1))
                nc.vector.tensor_scalar(stc(iRM), stc(iRSTD), stc(iMEAN),
                                        -1.0, OP.mult, OP.mult)
                rstd_b = spool.tile([P, 1], f32, tag="rstd_b",
                                    name=f"rstd{b}")
                rm_b = spool.tile([P, 1], f32, tag="rm_b", name=f"rm{b}")
                nc.gpsimd.partition_broadcast(rstd_b[:], stc(iRSTD))
                nc.gpsimd.partition_broadcast(rm_b[:], stc(iRM))
                d_b[b] = spool.tile([P, CT], f32, tag="d", name=f"d{b}")
                nc.vector.scalar_tensor_tensor(
                    d_b[b][:], wgam[:], rm_b[:, 0:1], wbet[:],
                    OP.mult, OP.add)
                wsc[b] = []
                for k in range(CT):
                    t = wscp.tile([P, C], bf16, tag="wsc", name=f"wsc{b}{k}")
                    nc.vector.tensor_scalar_mul(t[:], wtv(k),
                                                rstd_b[:, 0:1])
                    wsc[b].append(t)

                # ---------- phase 2 (pointwise + residual) ----------
                for oi in range(CT):
                    for h in range(TH):
                        o0 = h * HW_
                        ops = pspool.tile([P, 2048], f32, tag="ps",
                                          name=f"ops{b}{oi}{h}")
                        for k in range(CT):
                            for c0, wc in CHUNKS:
                                nc.tensor.matmul(
                                    ops[:, c0:c0 + wc],
                                    wsc[b][k][:, oi * P:(oi + 1) * P],
                                    pt[b, k][:, o0 + c0: o0 + c0 + wc],
                                    start=(k == 0), stop=(k == CT - 1))
                        yt = opool.tile([P, HW_], bf16, tag="y",
                                        name=f"y{b}{oi}{h}")
                        nc.vector.scalar_tensor_tensor(
                            yt[:], ops[:, 0:HW_], d_b[b][:, oi:oi + 1],
                            xt[b, oi][:, o0:o0 + HW_], OP.add, OP.add)
                        nc.sync.dma_start(
                            y_d[b, oi * P:(oi + 1) * P, o0:o0 + HW_], yt[:])

    nc.compile()
    return nc


def _host_prep(bn_gamma, bn_beta, bn_mean, bn_var, dw_w, gln_gamma, gln_beta,
               pw_w):
    import ml_dtypes
    f64 = np.float64
    bf = ml_dtypes.bfloat16
    s = bn_gamma.astype(f64) / np.sqrt(bn_var.astype(f64) + BN_EPS)
    bb = bn_beta.astype(f64) - bn_mean.astype(f64) * s
    w = dw_w[:, 0, :].astype(f64)                      # [C, 3]
    dg = np.zeros((CT * 3, P, P), np.float32)
    for ci in range(CT):
        sl = slice(ci * P, (ci + 1) * P)
        for k in range(3):
            dg[ci * 3 + k] = np.diag((s[sl] * w[sl, k])).astype(np.float32)
    s_safe = np.where(np.abs(s) < 1e-12, 1e-12, s)
    w0 = w[:, 0]
    w0c = np.where(np.abs(w0) < 1e-6, np.where(w0 < 0, -1e-6, 1e-6), w0)

    def pcol(v):
        return v.reshape(CT, P).T

    pads = pcol(-bb / s_safe)              # g pad: s*pad+bb = 0
    pads0 = pcol(-bb * w0c)                # t0 pad = pad*(s*w0c)
    bsum = pcol(bb * w.sum(1))
    w0s = pcol(s * w0c)
    w1s = pcol(s * w[:, 1])
    r2 = pcol(w[:, 2] / w0c)
    wtT = (pw_w.astype(f64) * gln_gamma.astype(f64)[None, :]).T   # [C, O]
    wt = np.ascontiguousarray(
        wtT.reshape(CT, P, C).transpose(1, 0, 2).reshape(P, CT * C)).astype(bf)
    wgam = pcol(pw_w.astype(f64) @ gln_gamma.astype(f64))
    wbet = pcol(pw_w.astype(f64) @ gln_beta.astype(f64))
    dgb = np.ascontiguousarray(
        dg.reshape(CT * 3, P, P).transpose(1, 0, 2).reshape(P, CT * 3 * P)
    ).astype(bf)
    cvb = np.concatenate([pads, pads0], axis=1).astype(bf)
    cvf = np.concatenate([bsum, w0s, w1s, r2, wgam, wbet],
                         axis=1).astype(np.float32)
    return dict(dg=dgb, wt=wt, cvb=np.ascontiguousarray(cvb),
                cvf=np.ascontiguousarray(cvf))


def _get_program(alpha: float, n_dve: int = N_DVE):
    key = (round(float(alpha), 9), n_dve)
    if key not in _CACHE:
        _CACHE[key] = _build(float(alpha), n_dve)
    return _CACHE[key]


def run(inputs: dict, trace: bool = False, n_dve: int = N_DVE):
    """Run on 8 cores; returns (y_full, BassKernelResults)."""
    import ml_dtypes
    from concourse.bass_utils import run_bass_kernel_spmd

    inputs = {k: np.asarray(v) for k, v in inputs.items()}
    x = np.ascontiguousarray(inputs["x"], dtype=np.float32).astype(
        ml_dtypes.bfloat16)
    alpha = float(np.asarray(inputs["prelu_a"]).reshape(-1)[0])
    consts = _host_prep(
        inputs["bn_gamma"], inputs["bn_beta"], inputs["bn_mean"],
        inputs["bn_var"], inputs["dw_w"], inputs["gln_gamma"],
        inputs["gln_beta"], inputs["pw_w"])
    nc = _get_program(alpha, n_dve)
    in_maps = [
        {"x": x[i * BPC:(i + 1) * BPC], **consts} for i in range(NCORES)
    ]
    res = run_bass_kernel_spmd(nc, in_maps, list(range(NCORES)), trace=trace)
    y = np.concatenate([res.results[i]["y"] for i in range(NCORES)], axis=0)
    return y.astype(np.float32), res


def kernel(**inputs) -> np.ndarray:
    y, _ = run(inputs, trace=False)
    return y
